# revision 23
# baseline (speedup 1.0000x reference)
"""AttentionBlock (GroupNorm -> QKV -> full attention -> out-proj + residual)
for B=4, C=128, N=4096 on 8 Trainium2 NeuronCores.

Sharding: 8 cores = 4 batches x 2 query-slabs of N/2. Every core runs the
same program; the host rolls each core's x so its query slab is always
columns [0, N/2).

Design (v3):
- q/k/v are never materialized. Scores: s = h^T M h + bias with
  M = w_q^T w_k composed on the host. The out-projection is composed into
  the v path on the host (wvo = (w_out @ w_v)^T), so attention directly
  produces wo^T(v P) and the epilogue is just normalize + bias + residual.
- The device computes GroupNorm stats (bn_stats on a host-shipped bf16
  copy of x) and folds the GN affine into the QK weights (wMA), the v
  weights (wvAB) and the biases on the fly.
- Attention runs in 4 passes of 512 queries. Each PSUM score slot
  [128, 1024] holds a PAIR of 128-key j-tiles x 512 queries, so one
  1024-wide exp covers a whole fp8 DoubleRow PV/rowsum pair and the
  per-pass accumulators (h2u, rowsum) are one PSUM bank each - that
  leaves THREE score slots, which is what keeps QK, exp and the int-trick
  pipeline overlapped.
- exp is split across engines by a per-pair type pattern: 'A'/'F' pairs
  run true exp on ACT; 'P'/'G' pairs compute exp on DVE via the
  Schraudolph int32 bit trick (tensor_scalar mult+add -> i32 = f32 bits)
  and convert bits->fp8 on GpSimd. Rowsums come from fp8 DoubleRow
  all-ones matmuls on the same P pairs.
- Pass 0 runs QK in f32r straight from h (types 'F'/'G'), so nothing
  waits on the fp8 repacks; passes 1-3 use fp8 DoubleRow QK on packed
  [64, 2, .] operands built by SBUF->SBUF DMA repacks (2x PE rate).
- finish (1/rowsum) + epilogue of pass k are deferred into pass k+1
  (recip/mult on DVE, residual add on GpSimd); only the last chunk's
  chain sits on the tail.
End-to-end relative error vs the fp32 reference ~3.6e-4; TimelineSim
per-core time ~80.5us (baseline f32r kernel: 94.0us).
"""

import math
import sys
from collections import deque

if "/opt/trn_rl_repo" not in sys.path:
    sys.path.insert(0, "/opt/trn_rl_repo")

import numpy as np

C = 128
G = 8
GS = C // G  # channels per group
EPS = 1e-5
N_CORES = 8
SCALE_C = None  # set in build from C


def default_pattern(NP=16, NPASS=4):
    """Per pair-tile (2 j-tiles x 512 queries) exp engine assignment, one
    char per pair-tile per pass. 'F' = f32r fast-path QK + ACT exp (first
    pass warmup); 'A' = ACT exp; 'P' = DVE int-trick + Pool fp8 convert.
    Totals: A 38 (incl 2 F), P 26."""
    p = []
    #        0123456789012345
    p.append("FFFGFGFGFGFGFAFF")  # pass 0: f32r QK; 10F+1A(packed-warm), 5G
    p.append("APAPAPAPAPAPAAPA")  # pass 1: 9A, 7P
    p.append("APAPAPAPAPAPAAPA")  # pass 2: 9A, 7P
    p.append("APAPAPAPAPAAPAAA")  # pass 3: 11A, 5P
    return "".join(p)


def build(N=4096, pattern=None, lag=4,
          fin_prev_t=(2, 4), tailw=(256, 256)):
    """Build the per-core Bass program. Returns the compiled Bacc module."""
    import concourse.bacc as bacc
    import concourse.bass as bass
    import concourse.mybir as mybir
    import concourse.tile as tile

    f32 = mybir.dt.float32
    f32r = mybir.dt.float32r
    bf16 = mybir.dt.bfloat16
    f8 = mybir.dt.float8e4
    i32 = mybir.dt.int32
    AF = mybir.ActivationFunctionType
    OP = mybir.AluOpType
    DR = mybir.MatmulPerfMode.DoubleRow

    S = N // 2           # query slab width per core
    ICW = 512            # i-chunk width per pass (h2u/rs = 1 PSUM bank each)
    NPASS = S // ICW     # 4 passes over i
    NJT = N // 128
    NP = NJT // 2        # pair-tiles per pass (one [C,1024] slot each)
    BNC = 512            # bn_stats chunk
    NBN = N // BNC
    HCW = 1024           # h8 chunk
    SCALE = 1.0 / math.sqrt(C)
    # Schraudolph exp constants: exp(x) ~ bitcast(int(x*K + B))
    CORR = 0.043677448
    K32 = SCALE * (1 << 23) / math.log(2.0)
    B32 = float((1 << 23) * (127 - CORR))

    if pattern is None:
        pattern = default_pattern(NP, NPASS)
    assert len(pattern) == NPASS * NP

    nc = bacc.Bacc("TRN2", target_bir_lowering=False, debug=False)

    xb_d = nc.dram_tensor("xb", [C, N], bf16, kind="ExternalInput").ap()
    xf_d = nc.dram_tensor("xf", [C, S], f32, kind="ExternalInput").ap()
    # wb = [gmask | M | wvo | bcat(4)]  (wvo = (w_out @ w_v).T, so the
    # out-projection is pre-composed into the v path)
    wb_d = nc.dram_tensor("wb", [C, 3 * C + 4], f32, kind="ExternalInput").ap()
    o_d = nc.dram_tensor("out", [C, S], f32, kind="ExternalOutput").ap()

    with tile.TileContext(nc) as tc:
        with tc.tile_pool(name="consts", bufs=1) as cp, \
             tc.tile_pool(name="big", bufs=1) as bp, \
             tc.tile_pool(name="small", bufs=3) as sp_, \
             tc.tile_pool(name="pP", bufs=9) as pP, \
             tc.tile_pool(name="pT", bufs=5) as pT, \
             tc.tile_pool(name="ps_sT", bufs=3, space="PSUM") as psT, \
             tc.tile_pool(name="ps_rs", bufs=1, space="PSUM") as prs, \
             tc.tile_pool(name="ps_h2", bufs=1, space="PSUM") as ph2:

            # ---- DMA loads (few, fat: HWDGE costs ~625ns per DMA) ----
            xB = bp.tile([C, N], bf16, tag="xB")
            XDC = 1024
            for dc in range(N // XDC):
                nc.sync.dma_start(xB[:, dc * XDC:(dc + 1) * XDC],
                                  xb_d[:, dc * XDC:(dc + 1) * XDC])
            wbS = cp.tile([C, 3 * C + 4], f32, tag="wb")
            nc.sync.dma_start(wbS[:], wb_d[:])
            mS = wbS[:, 0:C]
            wM = wbS[:, C:2 * C]
            wvo = wbS[:, 2 * C:3 * C]
            bS = wbS[:, 3 * C:3 * C + 4]

            # ---- constants ----
            onesF8_2 = cp.tile([C, 2, C], f8, tag="onesf82")
            nc.vector.memset(onesF8_2[:], 1.0)
            epsT = cp.tile([C, 1], f32, tag="eps")
            nc.vector.memset(epsT[:], EPS)
            # dummy act: trigger the exp table load early on ACT
            dumT = cp.tile([C, 1], f32, tag="dum")
            nc.scalar.activation(out=dumT[:], in_=epsT[:], func=AF.Exp, scale=1.0)

            # ---- big SBUF tensors ----
            h8 = bp.tile([C, N], f8, tag="h8")
            hRf = bp.tile([C, N], f32r, tag="hRf")         # f32r pass-0 keys
            qRf = bp.tile([C, ICW], f32r, tag="qRf")       # f32r fast-path qt
            ht2 = bp.tile([64, 2, N], f8, tag="ht2")       # packed QK stationary
            qt8 = bp.tile([C, S], f8, tag="qt8")
            qtt = bp.tile([64, 2, S], f8, tag="qtt")       # packed QK moving
            vTR = bp.tile([C, N], f8, tag="vT")            # [key, chan] (wo-folded)
            xfS = bp.tile([C, S], f32, tag="xf")
            outS = bp.tile([C, S], f32, tag="outS")

            wMA = cp.tile([C, C], bf16, tag="wMA")
            wvAB = cp.tile([C, C], bf16, tag="wvAB")
            aT = sp_.tile([C, 1], f32, tag="aT")
            bT = sp_.tile([C, 1], f32, tag="bT")
            dS = sp_.tile([C, 1], f32, tag="dS")
            beffT = sp_.tile([C, 1], f32, tag="beffT")

            # ================= preamble =================
            # GroupNorm stats on bf16 x
            st6 = sp_.tile([C, NBN, 6], f32, tag="st6")
            for i in range(NBN):
                nc.vector.bn_stats(out=st6[:, i, :],
                                   in_=xB[:, i * BNC:(i + 1) * BNC])
            mv = sp_.tile([C, 2], f32, tag="mv")
            nc.vector.bn_aggr(out=mv[:], in_=st6[:])
            nc.vector.scalar_tensor_tensor(out=mv[:, 1:2], in0=mv[:, 0:1],
                                           scalar=mv[:, 0:1], in1=mv[:, 1:2],
                                           op0=OP.mult, op1=OP.add)
            pre = psT.tile([C, 2 * ICW], f32, tag="sT", name="pre0")
            nc.tensor.matmul(pre[:, 0:2], mS, mv[:], start=True, stop=True)
            gst = sp_.tile([C, 2], f32, tag="gst")
            nc.vector.tensor_copy(gst[:], pre[:, 0:2])
            # xv = eps + gEx2 - gmean^2;  inv = sqrt(1/xv)
            gv = sp_.tile([C, 1], f32, tag="gv")
            nc.vector.scalar_tensor_tensor(out=gv[:], in0=gst[:, 0:1],
                                           scalar=gst[:, 0:1], in1=gst[:, 1:2],
                                           op0=OP.mult, op1=OP.subtract)
            xv = sp_.tile([C, 1], f32, tag="xv")
            nc.vector.tensor_tensor(out=xv[:], in0=epsT[:], in1=gv[:],
                                    op=OP.subtract)
            rxv = sp_.tile([C, 1], f32, tag="rxv")
            nc.vector.reciprocal(out=rxv[:], in_=xv[:])
            inv = sp_.tile([C, 1], f32, tag="inv")
            nc.scalar.activation(out=inv[:], in_=rxv[:], func=AF.Sqrt)
            nc.vector.tensor_tensor(out=aT[:], in0=bS[:, 2:3], in1=inv[:],
                                    op=OP.mult)
            nc.vector.tensor_tensor(out=bT[:], in0=gst[:, 0:1], in1=aT[:],
                                    op=OP.mult)
            nc.vector.tensor_tensor(out=bT[:], in0=bS[:, 3:4], in1=bT[:],
                                    op=OP.subtract)
            # folded weights
            nc.vector.tensor_scalar(out=wMA[:], in0=wM, scalar1=aT[:],
                                    scalar2=None, op0=OP.mult)
            nc.vector.tensor_scalar(out=wvAB[:], in0=wvo, scalar1=aT[:],
                                    scalar2=None, op0=OP.mult)
            # delta = M^T bT + bqt (qt bias); beff += wo@wv@bT = wvo^T bT
            pre2 = psT.tile([C, 2 * ICW], f32, tag="sT", name="pre1")
            nc.tensor.matmul(pre2[:, 0:1], wM, bT[:], start=True, stop=True)
            nc.vector.tensor_tensor(out=dS[:], in0=bS[:, 0:1], in1=pre2[:, 0:1],
                                    op=OP.add)
            nc.tensor.matmul(pre2[:, 4:5], wvo, bT[:], start=True, stop=True)
            nc.vector.tensor_tensor(out=beffT[:], in0=bS[:, 1:2],
                                    in1=pre2[:, 4:5], op=OP.add)

            # fast-path operands: hRf (keys 0:512 f32r), qRf (i 0:512 f32r)
            for rc in range(4):
                nc.vector.tensor_scalar(out=hRf[:, rc * HCW:(rc + 1) * HCW],
                                        in0=xB[:, rc * HCW:(rc + 1) * HCW],
                                        scalar1=aT[:], scalar2=bT[:],
                                        op0=OP.mult, op1=OP.add)

            def qt_mms(cc, dst, col0):
                slx = slice(cc * ICW, (cc + 1) * ICW)
                nc.tensor.matmul(dst[0:64, col0:col0 + ICW], wMA[:, 0:64],
                                 xB[:, slx], start=True, stop=True)
                nc.tensor.matmul(dst[64:128, col0:col0 + ICW], wMA[:, 64:128],
                                 xB[:, slx], start=True, stop=True)

            # qt chunks 0/1 (i-cols 0:1024): qRf (ACT, 512) + fp8 (DVE, 1024)
            qtp01 = psT.tile([C, 2 * ICW], f32, tag="sT", name="qtp01")
            qt_mms(0, qtp01, 0)
            qt_mms(1, qtp01, ICW)
            nc.scalar.activation(out=qRf[:], in_=qtp01[:, 0:ICW],
                                 func=AF.Identity, bias=dS[:], scale=1.0)
            nc.vector.tensor_scalar(out=qt8[:, 0:2 * ICW], in0=qtp01[:],
                                    scalar1=1.0, scalar2=dS[:],
                                    op0=OP.mult, op1=OP.add)
            nc.sync.dma_start(qtt[:, 0, 0:2 * ICW], qt8[0:64, 0:2 * ICW])
            nc.sync.dma_start(qtt[:, 1, 0:2 * ICW], qt8[64:128, 0:2 * ICW])

            # h8 = fp8(aT*xB + bT); repack halves as they complete
            for hc in range(N // HCW):
                sl = slice(hc * HCW, (hc + 1) * HCW)
                nc.vector.tensor_scalar(out=h8[:, sl], in0=xB[:, sl],
                                        scalar1=aT[:], scalar2=bT[:],
                                        op0=OP.mult, op1=OP.add)
                if hc % 2 == 1:
                    sl2 = slice((hc - 1) * HCW, (hc + 1) * HCW)
                    nc.sync.dma_start(ht2[:, 0, sl2], h8[0:64, sl2])
                    nc.sync.dma_start(ht2[:, 1, sl2], h8[64:128, sl2])

            # v-tilde (= wo^T v) chunks: mms into psT slots; chunk 0 on ACT
            def vt_mms(g, vtp):
                for jj in range(HCW // 128):
                    tj = (HCW // 128) * g + jj
                    nc.tensor.matmul(vtp[:, 128 * jj:128 * (jj + 1)],
                                     xB[:, 128 * tj:128 * (tj + 1)], wvAB[:],
                                     start=True, stop=True)

            vtp0 = psT.tile([C, 2 * ICW], f32, tag="sT", name="vtp0")
            vt_mms(0, vtp0)
            nc.scalar.activation(out=vTR[:, 0:HCW], in_=vtp0[:],
                                 func=AF.Identity, scale=1.0)

            # residual x (f32), single fat DMA, overlapped with attention
            nc.sync.dma_start(xfS[:], xf_d[:])
            xpb = bp.tile([C, S], f32, tag="xpb")
            for pc in range(2):
                nc.vector.tensor_scalar(out=xpb[:, pc * S // 2:(pc + 1) * S // 2],
                                        in0=xfS[:, pc * S // 2:(pc + 1) * S // 2],
                                        scalar1=1.0, scalar2=beffT[:],
                                        op0=OP.mult, op1=OP.add)

            # ================= attention =================
            # Each slot holds scores for a PAIR of j-tiles x ICW queries:
            # slot[:, 0:512] = j-tile 2p, slot[:, 512:1024] = j-tile 2p+1.
            # One 1024-wide exp/op1 consumes the slot; PV/rowsum are single
            # 512-wide fp8 DoubleRow matmuls into 1-bank accumulators.
            acc = {}
            pend = deque()

            def emit_pair(job):
                ps, p, ptile = job
                h2p, rsp = acc[ps]
                vpair = vTR[:, 256 * p:256 * (p + 1)].rearrange(
                    "p (two c) -> p two c", two=2)
                nc.tensor.matmul(h2p[:], vpair, ptile[:],
                                 start=p == 0, stop=p == NP - 1, perf_mode=DR)
                nc.tensor.matmul(rsp[:], onesF8_2[:], ptile[:],
                                 start=p == 0, stop=p == NP - 1, perf_mode=DR)

            def fin_chunk(ps, c0, cw):
                # out[:, i] = h2u[:, i]/rs[i] + beff + x
                h2p, rsp = acc[ps]
                sl_i = slice(ps * ICW + c0, ps * ICW + c0 + cw)
                sl_f = slice(c0, c0 + cw)
                recipB = sp_.tile([C, cw], f32, tag=f"recipB{cw}")
                nc.vector.reciprocal_approx_fast(out=recipB[:], in_=rsp[:, sl_f])
                nc.vector.tensor_tensor(out=outS[:, sl_i], in0=h2p[:, sl_f],
                                        in1=recipB[:], op=OP.mult)

            def epi_chunk(ps, c0, cw, dma, eng="pool"):
                sl_i = slice(ps * ICW + c0, ps * ICW + c0 + cw)
                if eng == "pool":
                    nc.gpsimd.tensor_tensor(out=outS[:, sl_i], in0=outS[:, sl_i],
                                            in1=xpb[:, sl_i], op=OP.add)
                else:
                    nc.vector.tensor_tensor(out=outS[:, sl_i], in0=outS[:, sl_i],
                                            in1=xpb[:, sl_i], op=OP.add)
                if dma:
                    d0, dw = dma
                    sl_d = slice(ps * ICW + d0, ps * ICW + d0 + dw)
                    nc.sync.dma_start(o_d[:, sl_d], outS[:, sl_d])

            # in-pass service schedule: (pass, pair) -> list of thunks
            def srv_vt(g, eng):
                def thunk():
                    spt = psT.tile([C, 2 * ICW], f32, tag="sT", name=f"vtp{g}")
                    vt_mms(g, spt)
                    if eng == "act":
                        nc.scalar.activation(out=vTR[:, HCW * g:HCW * (g + 1)],
                                             in_=spt[:], func=AF.Identity,
                                             scale=1.0)
                    else:
                        nc.vector.tensor_copy(vTR[:, HCW * g:HCW * (g + 1)],
                                              spt[:])
                return thunk

            def srv_qt_hi():
                def thunk():
                    spt = psT.tile([C, 2 * ICW], f32, tag="sT", name="qtp23")
                    qt_mms(2, spt, 0)
                    qt_mms(3, spt, ICW)
                    nc.scalar.activation(out=qt8[:, 2 * ICW:S], in_=spt[:],
                                         func=AF.Identity, bias=dS[:],
                                         scale=1.0)
                    nc.sync.dma_start(qtt[:, 0, 2 * ICW:S], qt8[0:64, 2 * ICW:S])
                    nc.sync.dma_start(qtt[:, 1, 2 * ICW:S],
                                      qt8[64:128, 2 * ICW:S])
                return thunk

            srv = {(0, 4): [srv_vt(1, "act")], (0, 7): [srv_vt(2, "act")],
                   (0, 10): [srv_vt(3, "act")], (0, 13): [srv_qt_hi()]}
            for ps in range(1, NPASS):
                for i, q in enumerate(fin_prev_t):
                    srv.setdefault((ps, q), []).append(
                        lambda ps=ps, i=i: (fin_chunk(ps - 1, i * 256, 256),
                                            epi_chunk(ps - 1, i * 256, 256,
                                                      (0, ICW) if i == 1
                                                      else None)))

            for ps in range(NPASS):
                acc[ps] = (ph2.tile([C, ICW], f32, tag="h2u", name=f"h2u{ps}"),
                           prs.tile([C, ICW], f32, tag="rs", name=f"rs{ps}"))
                ic0 = ps * ICW
                for p in range(NP):
                    ty = pattern[ps * NP + p]
                    for thunk in srv.get((ps, p), ()):
                        thunk()
                    # QK for j-tiles 2p, 2p+1 into one [C, 1024] slot
                    sT = psT.tile([C, 2 * ICW], f32, tag="sT")
                    for k in range(2):
                        t = 2 * p + k
                        ks = slice(k * ICW, (k + 1) * ICW)
                        if ty in "FG":
                            nc.tensor.matmul(
                                sT[:, ks], hRf[:, 128 * t:128 * (t + 1)],
                                qRf[:], start=True, stop=True)
                        else:
                            nc.tensor.matmul(
                                sT[:, ks], ht2[:, :, 128 * t:128 * (t + 1)],
                                qtt[:, :, ic0:ic0 + ICW],
                                start=True, stop=True, perf_mode=DR)
                    # exp: one 1024-wide op -> Ppair [C, 2, 512] fp8
                    Ppair = pP.tile([C, 2, ICW], f8, tag="P", name=f"P{ps}_{p}")
                    if ty in "AF":
                        nc.scalar.activation(out=Ppair[:], in_=sT[:],
                                             func=AF.Exp, scale=SCALE)
                    elif ty == "G":
                        ptmp = pT.tile([C, 2 * ICW], i32, tag="Ptmp")
                        nc.vector.tensor_scalar(out=ptmp[:], in0=sT[:],
                                                scalar1=K32, scalar2=B32,
                                                op0=OP.mult, op1=OP.add)
                        nc.gpsimd.tensor_copy(out=Ppair[:],
                                              in_=ptmp[:].bitcast(f32))
                    else:  # P
                        ptmp = pT.tile([C, 2 * ICW], i32, tag="Ptmp")
                        nc.vector.tensor_scalar(out=ptmp[:], in0=sT[:],
                                                scalar1=K32, scalar2=B32,
                                                op0=OP.mult, op1=OP.add)
                        nc.gpsimd.tensor_copy(out=Ppair[:],
                                              in_=ptmp[:].bitcast(f32))
                    pend.append((ps, p, Ppair))
                    mlag = 6 if (ps > 0 and p < 8) else lag
                    mlag = min(mlag, NP - 1 - p)
                    while len(pend) > mlag:
                        emit_pair(pend.popleft())
                while pend:
                    emit_pair(pend.popleft())

            # tail: last pass finish + epilogue
            c0 = 0
            for i, cw in enumerate(tailw):
                fin_chunk(NPASS - 1, c0, cw)
                epi_chunk(NPASS - 1, c0, cw, (c0, cw), eng="dve")
                c0 += cw
            assert c0 == ICW

    nc.compile()
    return nc


def host_inputs(x, gn_w, gn_b, w_qkv, b_qkv, w_out, b_out):
    """Build the 8 per-core input maps from the full problem inputs."""
    import ml_dtypes

    x = np.asarray(x, dtype=np.float32)
    B, _, N = x.shape
    S = N // 2
    w_qkv = np.asarray(w_qkv, np.float32)
    w_out = np.asarray(w_out, np.float32)
    b_qkv = np.asarray(b_qkv, np.float32)
    b_out = np.asarray(b_out, np.float32)
    gn_w = np.asarray(gn_w, np.float32)
    gn_b = np.asarray(gn_b, np.float32)

    # scores = h^T M h + h^T (M^T b + w_k^T b_q); q/k never materialized.
    # wvo composes the out-projection into the v path: v-tilde = wo^T v.
    M = w_qkv[0:C].T @ w_qkv[C:2 * C]
    wvo = (w_out @ w_qkv[2 * C:3 * C]).T
    gidx = np.arange(C) // GS
    gmask = (gidx[:, None] == gidx[None, :]).astype(np.float32) / GS
    bqt = w_qkv[C:2 * C].T @ b_qkv[0:C]
    b_eff = b_out + w_out @ b_qkv[2 * C:3 * C]
    bcat = np.stack([bqt, b_eff, gn_w, gn_b], axis=1)
    wb = np.concatenate([gmask, M, wvo, bcat], axis=1)
    wb = np.ascontiguousarray(wb, np.float32)           # [C, 3C+4]

    in_maps = []
    for core in range(N_CORES):
        b, half = divmod(core, 2)
        xb = np.roll(x[b], -half * S, axis=1)
        in_maps.append({
            "xb": np.ascontiguousarray(xb.astype(ml_dtypes.bfloat16)),
            "xf": np.ascontiguousarray(xb[:, :S]),
            "wb": wb})
    return in_maps


_NC_CACHE = {}
_RUNNER_CACHE = {}


def _make_runner(nc):
    """Compile-once runner: replicates bass2jax.run_bass_via_pjrt but keeps the
    jitted sharded callable so repeat executions skip recompilation."""
    import jax
    import concourse.mybir as mybir
    from jax.sharding import Mesh, PartitionSpec
    from jax.experimental.shard_map import shard_map
    from concourse.bass2jax import (_bass_exec_p, install_neuronx_cc_hook,
                                    partition_id_tensor)

    install_neuronx_cc_hook()
    partition_name = nc.partition_id_tensor.name if nc.partition_id_tensor else None
    in_names, out_names, out_avals, zero_shapes = [], [], [], []
    for alloc in nc.m.functions[0].allocations:
        if not isinstance(alloc, mybir.MemoryLocationSet):
            continue
        name = alloc.memorylocations[0].name
        if alloc.kind == "ExternalInput":
            if name == partition_name:
                continue
            in_names.append(name)
        elif alloc.kind == "ExternalOutput":
            out_names.append(name)
            shape = tuple(alloc.tensor_shape)
            dtype = mybir.dt.np(alloc.dtype)
            out_avals.append(jax.core.ShapedArray(shape, dtype))
            zero_shapes.append((shape, dtype))
    n_params = len(in_names)
    all_names = in_names + out_names
    if partition_name is not None:
        all_names = all_names + [partition_name]
    donate = tuple(range(n_params, n_params + len(out_names)))

    def _body(*args):
        operands = list(args)
        if partition_name is not None:
            operands.append(partition_id_tensor())
        return tuple(_bass_exec_p.bind(
            *operands, out_avals=tuple(out_avals), in_names=tuple(all_names),
            out_names=tuple(out_names), lowering_input_output_aliases=(),
            sim_require_finite=True, sim_require_nnan=True, nc=nc))

    devices = jax.devices()[:N_CORES]
    mesh = Mesh(np.asarray(devices), ("core",))
    specs = (PartitionSpec("core"),)
    sharded = jax.jit(
        shard_map(_body, mesh=mesh,
                  in_specs=specs * (n_params + len(out_names)),
                  out_specs=specs * len(out_names), check_rep=False),
        donate_argnums=donate, keep_unused=True)

    def run(in_maps):
        concat_in = [np.concatenate([np.asarray(m[nm]) for m in in_maps], axis=0)
                     for nm in in_names]
        concat_zeros = [np.zeros((N_CORES * s[0], *s[1:]), d) for s, d in zero_shapes]
        out_arrs = sharded(*concat_in, *concat_zeros)
        out_arrs = [np.asarray(a) for a in out_arrs]
        return [{nm: out_arrs[i].reshape(N_CORES, *out_avals[i].shape)[c]
                 for i, nm in enumerate(out_names)} for c in range(N_CORES)]

    return run


def get_runner(N=4096):
    if N not in _RUNNER_CACHE:
        if N not in _NC_CACHE:
            _NC_CACHE[N] = build(N)
        _RUNNER_CACHE[N] = _make_runner(_NC_CACHE[N])
    return _RUNNER_CACHE[N]


def kernel(x, gn_w, gn_b, w_qkv, b_qkv, w_out, b_out):
    from concourse._compat import axon_active

    x = np.asarray(x, dtype=np.float32)
    B, _, N = x.shape
    S = N // 2
    in_maps = host_inputs(x, gn_w, gn_b, w_qkv, b_qkv, w_out, b_out)
    if axon_active():
        results = get_runner(N)(in_maps)
    else:
        from concourse.bass_utils import run_bass_kernel_spmd

        if N not in _NC_CACHE:
            _NC_CACHE[N] = build(N)
        results = run_bass_kernel_spmd(_NC_CACHE[N], in_maps,
                                       core_ids=list(range(N_CORES))).results
    out = np.empty((B, C, N), dtype=np.float32)
    for core in range(N_CORES):
        b, half = divmod(core, 2)
        out[b, :, half * S:(half + 1) * S] = results[core]["out"]
    return out
